# revision 21
# baseline (speedup 1.0000x reference)
"""FAGCN layer on 8 Trainium2 NeuronCores (Bass/Tile).

Strategy (1D graph partition, dst-sharded):
  - Host: relabel nodes into 1568 degree-balanced windows of 64 slots
    (196 windows per core); bucket edges by dst window; split each
    window's edges by src slab (2 slabs reachable via int16 gather
    indices against base-biased table views); within each (window, slab)
    group edges are sorted by src table row so the gather sweeps HBM in
    ascending address order.
  - Launch 1 (dense, node-sharded): h = relu(x@w1T+b1), gate scalars
    a_dst/a_src = h@gwT (fused into the PE transpose via rhs=[I|gw]),
    norm = clip(deg,1)^-1/2. Emits (all writes partition-contiguous):
      aug  [NPC,256] f16, row p*NB+b = slot b*128+p: hn = norm*h (0:128),
           a_src (128); cols 129+ unwritten.
      own2 [NPC,130] f16, same permuted rows: EPS*h*sqrt(clip(deg,1))
           (0:128; the drain's norm-scale turns it into EPS*h),
           a_dst + gate_b (128).
      nrmW [64, NWC] f32 window-major: norm per dst slot.
  - Host: all-gather the f16 aug table (pure concatenation).
  - Launch 2 (edge phase, dst-sharded): per 128-edge chunk, dma_gather
    hn rows by src, round-robin over the 4 SWDGE queues (each queue is
    a different Q7 core pair, so descriptor generation pipelines 4-wide);
    per-edge gate built on the Scalar engine: sw = tanh(s3_onehot * t1)
    where t1 = a_src + a_dst + gate_b (a_dst broadcast via host-built
    one-hot matvec on the PE) — tanh(0)=0 keeps the one-hot zeros zero,
    so sw is the gate-scaled scatter matrix directly; scatter-add via
    matmul into PSUM per 64-dst window; the eps residual enters the PSUM
    chain as an identity matmul; drain applies norm[dst] on the Scalar
    engine. The DVE is kept nearly idle on purpose: its 2-port perf-mode
    ops lock GpSimd out of the shared SBUF port pair and starve SWDGE
    descriptor generation.
"""
import os
import sys

os.environ.setdefault("NEURON_RT_RESET_CORES", "1")

if "/opt/trn_rl_repo" not in sys.path:
    sys.path.insert(0, "/opt/trn_rl_repo")

import numpy as np

from concourse import bacc, bass, mybir, tile
from concourse.bass_utils import run_bass_kernel_spmd
from concourse.masks import make_identity

f32 = mybir.dt.float32
f16 = mybir.dt.float16
f8 = mybir.dt.float8e4
i16 = mybir.dt.int16
i32 = mybir.dt.int32
AF = mybir.ActivationFunctionType

N = 100000
E = 1600000
IN_DIM = 256
HID = 128
EPS = 0.3

NCORES = 8
WIN = 64
NWT = 1568            # total windows
NWC = NWT // NCORES   # 196 windows per core
NPC = NWC * WIN       # 12544 slots per core
NSLOT = NWT * WIN     # 100352 total slots
NB = NPC // 128       # 98 dense blocks per core
SB = 4                # windows per superblock
NSB = NWC // SB       # 49 superblocks per core
ASPLIT = 512          # slab A = the 512 lowest-aug-row edges of each window
EW = 256              # f16 elements per gather row (512B)
NQ = 4                # SWDGE queues (Q7 core pairs) for gathers

_prog_cache = {}
LAST_EXEC_NS = None  # [phase1_ns, phase2_ns] when KERNEL_TRACE=1


def _build_phase1():
    GB = 7  # blocks per write group (98 = 14*7)
    nc = bacc.Bacc(None)
    xT = nc.dram_tensor("xT", [2, 128, NPC], f16, kind="ExternalInput")
    degT = nc.dram_tensor("degT", [128, NB], f32, kind="ExternalInput")
    degW = nc.dram_tensor("degW", [WIN, NWC], f32, kind="ExternalInput")
    w1T = nc.dram_tensor("w1T", [2, 128, HID], f16, kind="ExternalInput")
    b1 = nc.dram_tensor("b1", [HID, 1], f32, kind="ExternalInput")
    gwT = nc.dram_tensor("gwT", [HID, 2], f16, kind="ExternalInput")
    gbc = nc.dram_tensor("gbc", [128, 1], f32, kind="ExternalInput")
    aug = nc.dram_tensor("aug", [NPC, EW], f16, kind="ExternalOutput")
    own2 = nc.dram_tensor("own2", [NPC, 130], f16, kind="ExternalOutput")
    nrmW = nc.dram_tensor("nrmW", [WIN, NWC], f32, kind="ExternalOutput")
    augv = aug.rearrange("(p b) e -> p b e", p=128)
    own2v = own2.rearrange("(p b) e -> p b e", p=128)

    with tile.TileContext(nc) as tc:
        with (
            tc.tile_pool(name="const", bufs=1) as cp,
            tc.tile_pool(name="sb", bufs=4) as sb,
            tc.tile_pool(name="wr", bufs=3) as wr,
            tc.tile_pool(name="ps", bufs=4, space="PSUM") as ps,
        ):
            identf = cp.tile([128, 128], f32)
            make_identity(nc, identf[:])
            idgw = cp.tile([128, 130], f16)
            nc.vector.tensor_copy(idgw[:, 0:128], identf[:])
            nc.sync.dma_start(out=idgw[:, 128:130], in_=gwT[:, :])
            xT_t = [cp.tile([128, NPC], f16, tag=f"xT{k}", name=f"xT{k}") for k in range(2)]
            for k in range(2):
                nc.sync.dma_start(out=xT_t[k][:], in_=xT[k])
            w1T_t = [cp.tile([128, HID], f16, tag=f"w1T{k}", name=f"w1T{k}") for k in range(2)]
            for k in range(2):
                nc.sync.dma_start(out=w1T_t[k][:], in_=w1T[k])
            b1_t = cp.tile([HID, 1], f32)
            nc.sync.dma_start(out=b1_t[:], in_=b1[:, :])
            gb_t = cp.tile([128, 1], f32)
            nc.sync.dma_start(out=gb_t[:], in_=gbc[:, :])
            degT_t = cp.tile([128, NB], f32)
            nc.sync.dma_start(out=degT_t[:], in_=degT[:, :])
            # per-block scalars for all blocks at once
            dcA = cp.tile([128, NB], f32)
            nc.vector.tensor_scalar_max(dcA[:], degT_t[:], 1.0)
            sqA = cp.tile([128, NB], f32)
            nc.scalar.sqrt(sqA[:], dcA[:])
            nrmA = cp.tile([128, NB], f32)
            nc.vector.reciprocal(nrmA[:], sqA[:])
            sq3A = cp.tile([128, NB], f32)
            nc.vector.tensor_scalar_mul(sq3A[:], sqA[:], EPS)
            # window-major norm table
            degW_t = cp.tile([WIN, NWC], f32)
            nc.sync.dma_start(out=degW_t[:], in_=degW[:, :])
            dcW = cp.tile([WIN, NWC], f32)
            nc.vector.tensor_scalar_max(dcW[:], degW_t[:], 1.0)
            sqW = cp.tile([WIN, NWC], f32)
            nc.scalar.sqrt(sqW[:], dcW[:])
            nrmW_t = cp.tile([WIN, NWC], f32)
            nc.vector.reciprocal(nrmW_t[:], sqW[:])
            nc.sync.dma_start(out=nrmW[:, :], in_=nrmW_t[:])

            for g0 in range(0, NB, GB):
                augb = wr.tile([128, GB, EW], f16, tag="augb")
                o2b = wr.tile([128, GB, 130], f16, tag="o2b")
                nc.vector.memset(augb[:, :, 129:EW], 0.0)
                nc.vector.memset(o2b[:, :, 129:130], 0.0)
                for j in range(GB):
                    blk = g0 + j
                    r0 = blk * 128
                    hT_ps = ps.tile([128, 128], f32, tag="hT_ps")
                    for k in range(2):
                        nc.tensor.matmul(
                            out=hT_ps[:], lhsT=w1T_t[k][:],
                            rhs=xT_t[k][:, r0 : r0 + 128],
                            start=(k == 0), stop=(k == 1),
                        )
                    hT = sb.tile([128, 128], f16, tag="hT")
                    nc.scalar.activation(
                        out=hT[:], in_=hT_ps[:], func=AF.Relu, bias=b1_t[:]
                    )
                    # transpose + gate projections in one matmul:
                    # cols 0:128 = h (slot-major), 128 = a_dst, 129 = a_src
                    hg_ps = ps.tile([128, 130], f32, tag="hg_ps")
                    nc.tensor.matmul(
                        out=hg_ps[:], lhsT=hT[:], rhs=idgw[:], start=True, stop=True
                    )

                    nc.vector.tensor_scalar_mul(
                        o2b[:, j, 0:128], hg_ps[:, 0:128], sq3A[:, blk : blk + 1]
                    )
                    nc.vector.tensor_tensor(
                        out=o2b[:, j, 128:129], in0=hg_ps[:, 128:129], in1=gb_t[:],
                        op=mybir.AluOpType.add,
                    )
                    nc.scalar.activation(
                        out=augb[:, j, 0:128], in_=hg_ps[:, 0:128], func=AF.Copy,
                        scale=nrmA[:, blk : blk + 1],
                    )
                    nc.vector.tensor_copy(augb[:, j, 128:129], hg_ps[:, 129:130])
                nc.sync.dma_start(out=augv[:, g0 : g0 + GB, :], in_=augb[:])
                nc.sync.dma_start(out=own2v[:, g0 : g0 + GB, :], in_=o2b[:])
    nc.finalize()
    return nc


def _build_phase2(CA, CB, BA, BB):
    import os
    NO_GATHER = os.environ.get("P2_NO_GATHER", "0") == "1"
    NO_MATVEC = os.environ.get("P2_NO_MATVEC", "0") == "1"
    NO_SW = os.environ.get("P2_NO_SW", "0") == "1"
    NO_SCATTER = os.environ.get("P2_NO_SCATTER", "0") == "1"
    K = CA + CB
    NIA = 128 * SB * CA
    NIB = 128 * SB * CB
    nc = bacc.Bacc(None, dynamic_dma_scratch_size=65536, num_swdge_queues=NQ)
    aug = nc.dram_tensor("aug", [NSLOT, EW], f16, kind="ExternalInput")
    own2 = nc.dram_tensor("own2", [NPC, 130], f16, kind="ExternalInput")
    nrmW = nc.dram_tensor("nrmW", [WIN, NWC], f32, kind="ExternalInput")
    ia = nc.dram_tensor("ia", [NSB, 128, NIA // 16], i16, kind="ExternalInput")
    ib = nc.dram_tensor("ib", [NSB, 128, NIB // 16], i16, kind="ExternalInput")
    s3 = nc.dram_tensor("s3", [NSB, 128, SB * K * WIN], f8, kind="ExternalInput")
    out = nc.dram_tensor("out", [WIN, NWC, HID], f16, kind="ExternalOutput")
    own2v = own2.rearrange("(p b) e -> p b e", p=128)

    qc = [0]  # gather queue round-robin counter

    with tile.TileContext(nc) as tc:
        with (
            tc.tile_pool(name="const", bufs=1) as cp,
            tc.tile_pool(name="gpool", bufs=4) as gp,
            tc.tile_pool(name="sbp", bufs=3) as sbp,
            tc.tile_pool(name="swp", bufs=4) as swp,
            tc.tile_pool(name="psz", bufs=2, space="PSUM") as psz,
            tc.tile_pool(name="psa", bufs=1, space="PSUM") as psa,
            tc.tile_pool(name="psg", bufs=2, space="PSUM") as psg,
        ):
            identf = cp.tile([128, 128], f32)
            make_identity(nc, identf[:])
            ident = cp.tile([128, 128], f16)
            nc.vector.tensor_copy(ident[:], identf[:])
            ones1 = cp.tile([1, 128], f16)
            nc.vector.memset(ones1[:], 1.0)

            for sbi in range(NSB):
                ita = sbp.tile([128, NIA // 16], i16, tag="ita")
                nc.sync.dma_start(out=ita[:], in_=ia[sbi])
                itb = sbp.tile([128, NIB // 16], i16, tag="itb")
                nc.sync.dma_start(out=itb[:], in_=ib[sbi])
                s3t = sbp.tile([128, SB * K * WIN], f8, tag="s3t")
                nc.sync.dma_start(out=s3t[:], in_=s3[sbi])
                own2t = sbp.tile([128, 2, 130], f16, tag="own2t")
                nc.sync.dma_start(out=own2t[:], in_=own2v[:, 2 * sbi : 2 * sbi + 2, :])
                nrmw = sbp.tile([WIN, SB], f32, tag="nrmw")
                nc.sync.dma_start(out=nrmw[:], in_=nrmW[:, sbi * SB : (sbi + 1) * SB])
                adT = psa.tile([1, 256], f32, tag="adT")
                for b in range(2):
                    nc.tensor.matmul(
                        out=adT[:, b * 128 : (b + 1) * 128],
                        lhsT=own2t[:, b, 128:129], rhs=ident[:],
                        start=True, stop=True,
                    )
                adS = sbp.tile([1, 256], f16, tag="adS")
                nc.scalar.activation(out=adS[:], in_=adT[:], func=AF.Copy)

                GA = gp.tile([128, SB * CA, EW], f16, tag="GA")
                GB = gp.tile([128, SB * CB, EW], f16, tag="GB")
                if NO_GATHER:
                    nc.vector.memset(GA[:], 0.0)
                    nc.vector.memset(GB[:], 0.0)
                else:
                    # per-call cap: 1024 idxs (best 4-queue interleave)
                    for Gt, view_base, it, L in (
                        (GA, BA, ita, NIA),
                        (GB, BB, itb, NIB),
                    ):  # noqa: BA/BB are per-build int16 view bases
                        o = 0
                        while o < L:
                            n = min(1024, L - o)
                            nc.gpsimd.dma_gather(
                                Gt[:, o // 128 : (o + n) // 128, :],
                                aug[view_base:, :],
                                it[:, o // 16 : (o + n) // 16],
                                n,
                                n,
                                EW,
                                queue_num=qc[0] % NQ,
                                single_packet=False,
                            )
                            qc[0] += 1
                            o += n

                otb = sbp.tile([WIN, SB, HID], f16, tag="otb")
                for wl in range(SB):
                    po = 64 * (wl % 2)
                    bh = wl // 2
                    # broadcast this window's a_dst row across all partitions
                    avbp = psa.tile([128, 1, WIN], f32, tag="avbp")
                    nc.tensor.matmul(
                        out=avbp[:, 0, :],
                        lhsT=ones1[:],
                        rhs=adS[:, bh * 128 + po : bh * 128 + po + WIN],
                        start=True,
                        stop=True,
                    )
                    # full (position, dstpos) tanh-arg grid, then mask by the
                    # (multi-)hot scatter pattern and tanh the whole window
                    argA = psg.tile([128, CA, WIN], f32, tag="argA")
                    nc.vector.tensor_tensor(
                        out=argA[:],
                        in0=GA[:, wl * CA : (wl + 1) * CA, 128:129].to_broadcast(
                            [128, CA, WIN]
                        ),
                        in1=avbp[:, 0:1, :].to_broadcast([128, CA, WIN]),
                        op=mybir.AluOpType.add,
                    )
                    argB = psg.tile([128, CB, WIN], f32, tag="argB")
                    nc.vector.tensor_tensor(
                        out=argB[:],
                        in0=GB[:, wl * CB : (wl + 1) * CB, 128:129].to_broadcast(
                            [128, CB, WIN]
                        ),
                        in1=avbp[:, 0:1, :].to_broadcast([128, CB, WIN]),
                        op=mybir.AluOpType.add,
                    )
                    swarg = sbp.tile([128, K * WIN], f16, tag="swarg")
                    w0 = wl * K * WIN
                    nc.vector.tensor_tensor(
                        out=swarg[:, 0 : CA * WIN],
                        in0=s3t[:, w0 : w0 + CA * WIN],
                        in1=argA[:],
                        op=mybir.AluOpType.mult,
                    )
                    nc.vector.tensor_tensor(
                        out=swarg[:, CA * WIN : K * WIN],
                        in0=s3t[:, w0 + CA * WIN : w0 + K * WIN],
                        in1=argB[:],
                        op=mybir.AluOpType.mult,
                    )
                    swt = swp.tile([128, K * WIN], f16, tag="swt")
                    nc.scalar.activation(out=swt[:], in_=swarg[:], func=AF.Tanh)

                    zp = psz.tile([WIN, HID], f32, tag="zp")
                    # residual: zp starts at EPS*h*sqrt(deg) for own slots
                    nc.tensor.matmul(
                        out=zp[:],
                        lhsT=ident[po : po + 64, po : po + 64],
                        rhs=own2t[po : po + 64, bh, 0:128],
                        start=True,
                        stop=False,
                    )
                    for c in range(K):
                        if c < CA:
                            Gx, col = GA, wl * CA + c
                        else:
                            Gx, col = GB, wl * CB + (c - CA)
                        sw = swt[:, c * WIN : (c + 1) * WIN]
                        if not NO_SCATTER:
                            nc.tensor.matmul(
                                out=zp[:],
                                lhsT=sw,
                                rhs=Gx[:, col, 0:128],
                                start=False,
                                stop=(c == K - 1),
                            )
                        elif c == K - 1:
                            nc.tensor.matmul(
                                out=zp[:],
                                lhsT=ident[po : po + 64, po : po + 64],
                                rhs=own2t[po : po + 64, bh, 0:128],
                                start=False,
                                stop=True,
                            )
                    nc.scalar.activation(
                        out=otb[:, wl, :], in_=zp[:], func=AF.Copy,
                        scale=nrmw[:, wl : wl + 1],
                    )
                nc.sync.dma_start(
                    out=out[:, sbi * SB : (sbi + 1) * SB, :], in_=otb[:]
                )
    nc.finalize()
    return nc


def _pack_idx16(flat_rel, n):
    """flat order i -> [i%16, i//16]; replicated 8x across 128 partitions."""
    t = np.zeros((16, n // 16), np.int16)
    t[np.arange(n) % 16, np.arange(n) // 16] = flat_rel
    return np.tile(t, (8, 1))


def _host_prep(edge_index):
    src = edge_index[0].astype(np.int64)
    dst = edge_index[1].astype(np.int64)
    deg = np.bincount(dst, minlength=N).astype(np.int64)

    # degree-balanced window assignment (iterative LPT)
    order = np.argsort(-deg, kind="stable")
    win_of_node = np.empty(N, np.int64)
    pos_of_node = np.empty(N, np.int64)
    load = np.zeros(NWT, np.int64)
    cnt = np.zeros(NWT, np.int64)
    for r0 in range(0, N, NWT):
        grp = order[r0 : r0 + NWT]
        bins = np.argsort(load, kind="stable")[: len(grp)]
        win_of_node[grp] = bins
        pos_of_node[grp] = cnt[bins]
        cnt[bins] += 1
        load[bins] += deg[grp]

    slot_of_node = win_of_node * WIN + pos_of_node
    orig_of_slot = np.full(NSLOT, -1, np.int64)
    orig_of_slot[slot_of_node] = np.arange(N)

    # per-edge attributes; src addressed by permuted aug row (p*NB + b)
    ew = win_of_node[dst]                # window
    es = slot_of_node[src]               # src slot
    lcl = es % NPC
    ar = (es // NPC) * NPC + (lcl % 128) * NB + lcl // 128   # aug row
    edl = pos_of_node[dst]               # dst position in window

    # sort by (window, aug row); slab A = each window's ASPLIT lowest-row
    # edges (rank split -> exactly 4+4 chunks per window, ~0 padding), and
    # the per-group gather reads HBM in ascending address order
    sidx = np.lexsort((ar, ew))
    ew_s = ew[sidx]
    ar_s = ar[sidx]
    edl_s = edl[sidx]
    wcnt = np.bincount(ew_s, minlength=NWT)
    wstart = np.zeros(NWT + 1, np.int64)
    wstart[1:] = np.cumsum(wcnt)
    rank = np.arange(E) - wstart[ew_s]
    eslab_s = (rank >= ASPLIT).astype(np.int64)
    qpos = np.where(eslab_s == 0, rank, rank - ASPLIT)

    cntsA = np.minimum(wcnt, ASPLIT)
    cntsB = wcnt - cntsA
    CA = int(np.ceil(cntsA.max() / 128))
    CB = int(np.ceil(max(int(cntsB.max()), 1) / 128))
    K = CA + CB

    # int16 view bases for the two slabs, derived from the data
    lastA = wstart[:NWT] + cntsA - 1
    okA = cntsA > 0
    tmaxA = int(ar_s[lastA[okA]].max())
    BA = max(tmaxA - 32767, 0)
    assert BA <= 32768, f"slab-A base {BA} out of int16 reach"
    BB = NSLOT - 1 - 32767
    okB = cntsB > 0
    if okB.any():
        tminB = int(ar_s[wstart[:NWT][okB] + ASPLIT].min())
        assert tminB - BB >= -32768, f"slab-B span too wide ({tminB} vs {BB})"

    core_s = ew_s // NWC
    wloc = ew_s % NWC
    sb_s = wloc // SB
    wl_s = wloc % SB
    p_s = qpos % 128
    c_loc = qpos // 128
    cchunk = np.where(eslab_s == 0, c_loc, CA + c_loc)
    j_s = wl_s * K + cchunk
    relidx = np.where(eslab_s == 0, ar_s - BA, ar_s - BB)
    assert relidx.min() >= -32768 and relidx.max() <= 32767

    NIA = 128 * SB * CA
    NIB = 128 * SB * CB
    per_core = []
    for c in range(NCORES):
        m = core_s == c
        sbv, wlv, pv, jv = sb_s[m], wl_s[m], p_s[m], j_s[m]
        slabv, relv, dlv, qv = eslab_s[m], relidx[m], edl_s[m], qpos[m]

        dl_arr = np.full((NSB, 128, SB * K), -1, np.int64)
        dl_arr[sbv, pv, jv] = dlv

        fa = np.zeros((NSB, NIA), np.int64)   # rel idx 0 = valid pad row
        fb = np.zeros((NSB, NIB), np.int64)
        mA = slabv == 0
        fa[sbv[mA], wlv[mA] * CA * 128 + qv[mA]] = relv[mA]
        mB = ~mA
        fb[sbv[mB], wlv[mB] * CB * 128 + qv[mB]] = relv[mB]

        # The Q7 gather drops a call's trailing run of negative indices, so
        # every call's final slot must hold a non-negative index. Swap a
        # non-negative slot from the same (window, slab) group into each
        # static call-tail position.
        def _fix_tails(f, C, joff):
            L = f.shape[1]
            tails = [min(o + 1024, L) - 1 for o in range(0, L, 1024)]
            span = 128 * C
            for s in range(NSB):
                for t in tails:
                    if f[s, t] >= 0:
                        continue
                    wg = t // span
                    g0 = wg * span
                    seg = f[s, g0 : g0 + span]
                    cand = np.nonzero(seg >= 0)[0]
                    assert cand.size, "all-negative gather group"
                    u = g0 + int(cand[-1])
                    p_t, c_t = t % 128, (t // 128) % C
                    p_u, c_u = u % 128, (u // 128) % C
                    j_t = wg * K + joff + c_t
                    j_u = wg * K + joff + c_u
                    f[s, t], f[s, u] = f[s, u], f[s, t]
                    tmp = dl_arr[s, p_t, j_t]
                    dl_arr[s, p_t, j_t] = dl_arr[s, p_u, j_u]
                    dl_arr[s, p_u, j_u] = tmp

        _fix_tails(fa, CA, 0)
        _fix_tails(fb, CB, CA)

        # (multi-)hot scatter table built from final dl
        si, pi, ji = np.nonzero(dl_arr >= 0)
        di = dl_arr[si, pi, ji]
        s3_arr = np.zeros((NSB, 128, SB * K * WIN), mybir.dt.np(f8))
        s3_arr[si, pi, ji * WIN + di] = 1.0

        ia_arr = np.stack([_pack_idx16(fa[s], NIA) for s in range(NSB)])
        ib_arr = np.stack([_pack_idx16(fb[s], NIB) for s in range(NSB)])
        per_core.append(dict(ia=ia_arr, ib=ib_arr, s3=s3_arr))

    return dict(
        deg=deg, orig_of_slot=orig_of_slot, slot_of_node=slot_of_node,
        CA=CA, CB=CB, BA=BA, BB=BB, per_core=per_core,
    )


def _run_spmd(nc, in_maps, trace):
    try:
        return run_bass_kernel_spmd(nc, in_maps, list(range(NCORES)), trace=trace)
    except Exception:
        # transient NRT exec-unit wedge: one retry after the implicit core reset
        return run_bass_kernel_spmd(nc, in_maps, list(range(NCORES)), trace=trace)


def kernel(x, edge_index, w1, b1, gate_w, gate_b):
    x = np.asarray(x, np.float32)
    edge_index = np.asarray(edge_index)
    w1 = np.asarray(w1, np.float32)
    b1 = np.asarray(b1, np.float32)
    gate_w = np.asarray(gate_w, np.float32)
    gate_b = np.asarray(gate_b, np.float32)

    prep = _host_prep(edge_index)
    CA, CB = prep["CA"], prep["CB"]
    orig_of_slot = prep["orig_of_slot"]
    deg = prep["deg"]

    # per-slot x / deg (zeros for empty slots)
    x_slots = np.zeros((NSLOT, IN_DIM), np.float32)
    deg_slots = np.zeros(NSLOT, np.float32)
    valid = orig_of_slot >= 0
    x_slots[valid] = x[orig_of_slot[valid]]
    deg_slots[valid] = deg[orig_of_slot[valid]]

    if "p1" not in _prog_cache:
        _prog_cache["p1"] = _build_phase1()
    nc1 = _prog_cache["p1"]

    w1T = np.ascontiguousarray(w1.T.reshape(2, 128, HID).astype(np.float16))
    b1c = np.ascontiguousarray(b1[:, None])
    gwT = np.ascontiguousarray(gate_w.reshape(2, HID).T.astype(np.float16))
    gbc = np.full((128, 1), float(gate_b[0]), np.float32)
    in_maps1 = []
    for c in range(NCORES):
        dsc = deg_slots[c * NPC : (c + 1) * NPC]
        in_maps1.append(dict(
            xT=np.ascontiguousarray(
                x_slots[c * NPC : (c + 1) * NPC].T.reshape(2, 128, NPC)
            ).astype(np.float16),
            degT=np.ascontiguousarray(dsc.reshape(NB, 128).T),
            degW=np.ascontiguousarray(dsc.reshape(NWC, WIN).T),
            w1T=w1T, b1=b1c, gwT=gwT, gbc=gbc,
        ))
    do_trace = os.environ.get("KERNEL_TRACE", "0") == "1"
    global LAST_EXEC_NS
    LAST_EXEC_NS = [None, None]
    br1 = _run_spmd(nc1, in_maps1, do_trace)
    r1 = br1.results
    LAST_EXEC_NS[0] = br1.exec_time_ns

    aug_full = np.concatenate([r1[c]["aug"] for c in range(NCORES)], axis=0)

    BA, BB = prep["BA"], prep["BB"]
    key2 = ("p2", CA, CB, BA, BB)
    if key2 not in _prog_cache:
        _prog_cache[key2] = _build_phase2(CA, CB, BA, BB)
    nc2 = _prog_cache[key2]

    in_maps2 = [
        dict(
            aug=aug_full,
            own2=r1[c]["own2"],
            nrmW=r1[c]["nrmW"],
            ia=prep["per_core"][c]["ia"],
            ib=prep["per_core"][c]["ib"],
            s3=prep["per_core"][c]["s3"],
        )
        for c in range(NCORES)
    ]
    br2 = _run_spmd(nc2, in_maps2, do_trace)
    r2 = br2.results
    LAST_EXEC_NS[1] = br2.exec_time_ns

    # out is [WIN, NWC, HID] window-major per core -> slot-ordered rows
    out_slots = np.concatenate(
        [
            r2[c]["out"].transpose(1, 0, 2).reshape(NPC, HID).astype(np.float32)
            for c in range(NCORES)
        ],
        axis=0,
    )
    result = np.empty((N, HID), np.float32)
    result[orig_of_slot[valid]] = out_slots[valid]
    return result
